# revision 19
# baseline (speedup 1.0000x reference)
"""GAT layer (N=8192, F_IN=256, H=64 per head, K=8 heads) on 8 Trainium2 cores.

Strategy (row-sharding, fully data-parallel, no collectives):
  reference per head k:
    h   = features @ W[k]                      [N, H]
    wh1 = h @ a[k,:H]; wh2 = h @ a[k,H:]       [N]
    e   = leaky_relu(wh1[:,None] + wh2[None,:], 0.2)
    att = softmax(where(adj>0, e, -9e15), axis=1)
    out = elu(att @ h)

  Algebra: with s = wh1[i] + wh2[j], G1 = exp(0.8 wh1), G2 = exp(0.8 wh2),
  X2 = exp(wh2):
    exp(lrelu(s)) / exp(0.2 wh1) = max(G1_i, 1/G2_j) * X2_j
  The row factor exp(0.2 wh1) cancels in softmax.  X2_j is folded into the
  value matrix ON THE HOST: h2 = [h * X2 | X2] (the last column produces the
  softmax denominator).  adj in {0,1} multiplies exactly:
    u[j,i]   = max(G1_i, invG2_j) * adj[j,i]
    acc[c,i] = sum_j h2[j,c] * u[j,i]            (PE, PSUM accumulate over j)
    out[i]   = elu(acc[0:H,i] / acc[H,i])

  Per-device work per j-group g (128 j's) within a 4-head sweep:
    2x DVE tensor_scalar   z_k = max(g1b_k, invG2_k[p])          (~200 ns)
    2x ACT activation      z_k = relu(g1b_k - invG2_k[p])        (~1.1 us,
                           parallel engine; exact PE correction below)
    1x DVE fused mask      u[s] = z[s] * adj  over all 4 heads   (~2.2 us,
                           stride-0 broadcast adjacency, ONE instruction)
    8x PE matmul [65,512]  main accumulation, both i-blocks
    4x PE matmul [65,512]  ACT-head corrections  acc += (h2*invG2)^T @ adj
  Batching the mask into one instruction (and keeping per-instruction
  semaphore overhead low) is worth ~2x on hardware vs per-head ops.
  Two 4-head sweeps over the adjacency (PSUM holds 8 [65,512] f32 accs).
  h2/g1b/invG2 come from tiny host matmuls (O(N K F) host work).

Per-core layout ([j,i]-transposed tiles so contraction j sits on partitions):
  adjr [128, 64, 2, 512] bf16 : adjr[p,g,ib,i] = adj[r0+ib*512+i, g*128+p]
  g1b  [128, 8, 2, 512] bf16  : G1 rows broadcast across partitions
  ig2  [128, 8, 64] f32       : invG2[p,k,g] = exp(-0.8 wh2)[g*128+p, k]
  nig2 [128, 8, 64] f32       : -invG2 (ACT bias)
  h2   [128, 64, 8, 65] bf16  : value matrix, X2-folded + den column
  h2c  [128, 64, 4, 65] bf16  : h2 * invG2 for the ACT heads (1,3,5,7)
"""

import sys
import os

sys.path.insert(0, "/opt/trn_rl_repo")

import numpy as np
import ml_dtypes
from contextlib import ExitStack

import concourse.bass as bass
import concourse.tile as tile
from concourse import bacc, mybir
from concourse.bass_utils import run_bass_kernel_spmd

N = 8192
F_IN = 256
H = 64
K = 8
ALPHA = 0.2
N_CORES = 8
R = N // N_CORES          # 1024 rows per core
IB = 2                    # i-blocks per core (512 columns of out-rows each)
IW = R // IB              # 512, i-width per block
G = N // 128              # 64 j-groups of 128
G_SUB = 4                 # j-groups per adjacency DMA
HA = H + 1                # 65: head value columns + denominator column
HC = G // 4               # 16 j-groups per h2 DMA chunk

SWEEPS = ((0, 1, 2, 3), (4, 5, 6, 7))
ACT_SLOTS = (1, 3)        # per-sweep slots whose z runs on the ACT engine
ACT_HEADS = tuple(sw[s] for sw in SWEEPS for s in ACT_SLOTS)  # (1,3,5,7)

F32 = mybir.dt.float32
BF16 = mybir.dt.bfloat16
AX = mybir.AluOpType

_cached = {}


def build_program(loop_t=1, n_act=len(ACT_SLOTS), pool_mask=False, pool_z=False, skew=True, u_bufs=3, z_bufs=3, gsub=None):
    key = (loop_t, n_act, pool_mask, pool_z, skew, u_bufs, z_bufs, gsub)
    if key in _cached:
        return _cached[key]

    nc = bacc.Bacc("TRN2", target_bir_lowering=False, debug=False,
                   num_devices=N_CORES)

    adjr_d = nc.dram_tensor("adjr", [128, G, IB, IW], BF16, kind="ExternalInput").ap()
    g1b_d = nc.dram_tensor("g1b", [128, K, IB, IW], BF16, kind="ExternalInput").ap()
    ig2_d = nc.dram_tensor("ig2", [128, K, G], F32, kind="ExternalInput").ap()
    nig2_d = nc.dram_tensor("nig2", [128, K, G], F32, kind="ExternalInput").ap()
    h2_d = nc.dram_tensor("h2", [128, G, K, HA], BF16, kind="ExternalInput").ap()
    h2c_d = nc.dram_tensor("h2c", [128, G, 4, HA], BF16, kind="ExternalInput").ap()
    out_d = nc.dram_tensor("out", [R, K * H], F32, kind="ExternalOutput").ap()

    PAIR = IB * IW            # 1024
    act_slots = ACT_SLOTS[:n_act]
    gsub = gsub or G_SUB

    with tile.TileContext(nc) as tc:
        with ExitStack() as ctx:
            const = ctx.enter_context(tc.tile_pool(name="const", bufs=1))
            adj_pool = ctx.enter_context(tc.tile_pool(name="adj", bufs=2))
            z_pool = ctx.enter_context(tc.tile_pool(name="z", bufs=z_bufs))
            u_pool = ctx.enter_context(tc.tile_pool(name="u", bufs=u_bufs))
            stg_pool = ctx.enter_context(tc.tile_pool(name="stg", bufs=2))
            fin_pool = ctx.enter_context(tc.tile_pool(name="fin", bufs=2))
            acc_sb_pool = ctx.enter_context(tc.tile_pool(name="accsb", bufs=2))
            psum = ctx.enter_context(tc.tile_pool(name="psum", bufs=8, space="PSUM"))

            # ---- constants ----
            g1b_sb = const.tile([128, K, IB, IW], BF16)
            nc.sync.dma_start(g1b_sb[:], g1b_d[:])
            ig2_sb = const.tile([128, K, G], F32)
            nc.sync.dma_start(ig2_sb[:], ig2_d[:])
            nig2_sb = const.tile([128, K, G], F32)
            nc.sync.dma_start(nig2_sb[:], nig2_d[:])
            ident = const.tile([128, 128], F32)
            from concourse.masks import make_identity
            make_identity(nc, ident[:])

            h2_sb = []
            h2c_sb = []
            for c in range(4):
                t = const.tile([128, HC, K, HA], BF16, name=f"h2_{c}")
                nc.sync.dma_start(t[:], h2_d[:, c * HC:(c + 1) * HC, :, :])
                h2_sb.append(t)
                if act_slots:
                    t2 = const.tile([128, HC, 4, HA], BF16, name=f"h2c_{c}")
                    nc.sync.dma_start(t2[:], h2c_d[:, c * HC:(c + 1) * HC, :, :])
                    h2c_sb.append(t2)

            loop_cm = tc.For_i(0, loop_t, 1) if loop_t > 1 else None
            if loop_cm is not None:
                ctx.enter_context(loop_cm)

            for si, sweep_heads in enumerate(SWEEPS):
                accs = {}
                for k in sweep_heads:
                    for ib in range(IB):
                        accs[(k, ib)] = psum.tile(
                            [HA, IW], F32, tag="ps", name=f"acc{k}_{ib}")
                def g1b_of(k):
                    return g1b_sb[:, k, :, :].rearrange("p b i -> p (b i)")

                def emit_z(dst, s, k, g):
                    if s in act_slots:
                        nc.scalar.activation(
                            dst, g1b_of(k),
                            mybir.ActivationFunctionType.Relu,
                            bias=nig2_sb[:, k, g:g + 1])
                    else:
                        eng = nc.gpsimd if (pool_z and s == 0) else nc.vector
                        eng.tensor_scalar(
                            dst, g1b_of(k),
                            ig2_sb[:, k, g:g + 1], None, op0=AX.max)

                def emit_mask_mm(g, z_all, adj_t, gi):
                    adj_pair = adj_t[:, gi, :, :].rearrange("p b i -> p (b i)")
                    u_all = u_pool.tile([128, 4, PAIR], BF16, tag="u")
                    adj_bc = adj_pair.unsqueeze(1).broadcast_to((128, 4, PAIR))
                    nc.vector.tensor_tensor(u_all[:], z_all[:], adj_bc,
                                            op=AX.mult)
                    for s, k in enumerate(sweep_heads):
                        lhs = h2_sb[g // HC][:, g % HC, k, :]
                        last = (g == G - 1) and s not in act_slots
                        for ib in range(IB):
                            nc.tensor.matmul(
                                accs[(k, ib)][:],
                                lhs,
                                u_all[:, s, ib * IW:(ib + 1) * IW],
                                start=(g == 0),
                                stop=last,
                            )
                    # exact corrections for the ACT heads:
                    # acc += (h2*invG2)^T @ adj
                    for s in act_slots:
                        k = sweep_heads[s]
                        ci = si * len(ACT_SLOTS) + ACT_SLOTS.index(s)
                        lhsc = h2c_sb[g // HC][:, g % HC, ci, :]
                        for ib in range(IB):
                            nc.tensor.matmul(
                                accs[(k, ib)][:],
                                lhsc,
                                adj_t[:, gi, ib, :],
                                start=False,
                                stop=(g == G - 1),
                            )

                pending = None   # (g, z_all, adj_t, gi) awaiting mask+mm
                for gs in range(G // gsub):
                    adj_t = adj_pool.tile([128, gsub, IB, IW], BF16)
                    nc.sync.dma_start(
                        adj_t[:], adjr_d[:, gs * gsub:(gs + 1) * gsub, :, :]
                    )
                    for gi in range(gsub):
                        g = gs * gsub + gi
                        z_all = z_pool.tile([128, 4, PAIR], BF16, tag="z")
                        for s, k in enumerate(sweep_heads):
                            emit_z(z_all[:, s, :], s, k, g)
                        if skew:
                            if pending is not None:
                                emit_mask_mm(*pending)
                            pending = (g, z_all, adj_t, gi)
                        else:
                            emit_mask_mm(g, z_all, adj_t, gi)
                if pending is not None:
                    emit_mask_mm(*pending)
                    pending = None
                # ---- drain this sweep ----
                sw = list(sweep_heads)
                NC = IW // 128
                for ib in range(IB):
                    stg = stg_pool.tile([128, 4, NC, HA], F32, tag="stg",
                                        name=f"stg{sw[0]}_{ib}")
                    for s, k in enumerate(sw):
                        acc_sb = acc_sb_pool.tile([HA, IW], F32, tag="accsb")
                        nc.scalar.copy(acc_sb[:], accs[(k, ib)][:])
                        pst4 = psum.tile([128, NC, HA], F32, tag="ps",
                                         name=f"pst{k}_{ib}")
                        for c in range(NC):
                            nc.tensor.transpose(
                                pst4[:, c, :], acc_sb[:, c * 128:(c + 1) * 128],
                                ident[0:HA, 0:HA],
                            )
                        nc.scalar.copy(stg[:, s, :, :], pst4[:])
                    for c in range(NC):
                        stgc = stg[:, :, c, :]
                        recips = fin_pool.tile([128, 4], F32, tag="recip")
                        nc.vector.reciprocal(recips[:], stgc[:, :, H])
                        fin = fin_pool.tile([128, 4, H], F32, tag="fin")
                        nc.vector.tensor_tensor(
                            fin[:], stgc[:, :, 0:H],
                            recips[:].unsqueeze(2).broadcast_to((128, 4, H)),
                            op=AX.mult,
                        )
                        # elu(x) = exp(min(x,0)) + (max(x,0) - 1)
                        fin2 = fin_pool.tile([128, 4 * H], F32, tag="fin2")
                        finf = fin[:].rearrange("p k f -> p (k f)")
                        nc.vector.tensor_scalar(
                            fin2[:], finf, 0.0, None, op0=AX.min
                        )
                        ex = fin_pool.tile([128, 4 * H], F32, tag="ex")
                        nc.scalar.activation(
                            ex[:], fin2[:], mybir.ActivationFunctionType.Exp
                        )
                        rel = fin_pool.tile([128, 4 * H], F32, tag="rel")
                        nc.vector.tensor_scalar(
                            rel[:], finf, 0.0, -1.0, op0=AX.max, op1=AX.add
                        )
                        res = fin_pool.tile([128, 4 * H], F32, tag="res")
                        nc.vector.tensor_tensor(res[:], ex[:], rel[:], op=AX.add)
                        resv = res[:].rearrange("p (k f) -> p k f", k=4)
                        nc.sync.dma_start(
                            out_d[ib * IW + c * 128:
                                  ib * IW + (c + 1) * 128,
                                  sw[0] * H:(sw[-1] + 1) * H],
                            resv[:],
                        )

    nc.compile()
    _cached[key] = nc
    return nc


def prepare_inputs(features, adj, W, a):
    """Host-side prep: tiny projections + per-core sharded/transposed layouts."""
    features = np.asarray(features, dtype=np.float32)
    adj = np.asarray(adj, dtype=np.float32)
    W = np.asarray(W, dtype=np.float32)
    a = np.asarray(a, dtype=np.float32)

    # av[k] = W[k] @ a[k]  -> wh = features @ av.T   (tiny: K*F_IN*H flops)
    av1 = np.einsum("kfh,kh->kf", W, a[:, :H])          # [K, F_IN]
    av2 = np.einsum("kfh,kh->kf", W, a[:, H:])          # [K, F_IN]
    wh1 = features @ av1.T                               # [N, K]
    wh2 = features @ av2.T                               # [N, K]
    G1 = np.exp(0.8 * wh1).astype(np.float32)            # row factors
    IG2 = np.exp(-0.8 * wh2).astype(np.float32)          # 1/G2 col factors
    X2 = np.exp(wh2).astype(np.float32)                  # folded into values

    # h2[j, k, :] = [h_k[j] * X2[j,k] | X2[j,k]]
    h2full = np.empty((N, K, HA), dtype=np.float32)
    for k in range(K):
        hk = features @ W[k]                             # [N, H]
        h2full[:, k, 0:H] = hk * X2[:, k:k + 1]
        h2full[:, k, H] = X2[:, k]
    h2cfull = np.empty((N, 4, HA), dtype=np.float32)
    for ci, k in enumerate(ACT_HEADS):
        h2cfull[:, ci, :] = h2full[:, k, :] * IG2[:, k:k + 1]
    # [128, G, K, HA], partition p = j % 128 within group g = j // 128
    h2 = np.ascontiguousarray(
        h2full.reshape(G, 128, K, HA).transpose(1, 0, 2, 3)
    ).astype(ml_dtypes.bfloat16)
    h2c = np.ascontiguousarray(
        h2cfull.reshape(G, 128, 4, HA).transpose(1, 0, 2, 3)
    ).astype(ml_dtypes.bfloat16)

    # ig2[p,k,g] = IG2[g*128+p, k]
    ig2 = np.ascontiguousarray(
        IG2.reshape(G, 128, K).transpose(1, 2, 0))       # [128, K, G]
    nig2 = np.ascontiguousarray(-ig2)

    in_maps = []
    for c in range(N_CORES):
        r0 = c * R
        # adjr[p, g, ib, i] = adj[r0 + ib*IW + i, g*128 + p]
        blk = adj[r0:r0 + R, :]                          # [R, N]
        adj_r = np.ascontiguousarray(
            blk.reshape(IB, IW, G, 128).transpose(3, 2, 0, 1)
        ).astype(ml_dtypes.bfloat16)                     # [128, G, IB, IW]
        # g1b[p, k, ib, i] = G1[r0 + ib*IW + i, k]
        g1_blk = G1[r0:r0 + R, :].reshape(IB, IW, K).transpose(2, 0, 1)
        g1b = np.broadcast_to(
            g1_blk[None].astype(ml_dtypes.bfloat16), (128, K, IB, IW))
        g1b = np.ascontiguousarray(g1b)
        in_maps.append({
            "adjr": adj_r,
            "g1b": g1b,
            "ig2": ig2,
            "nig2": nig2,
            "h2": h2,
            "h2c": h2c,
        })
    return in_maps


def kernel(features, adj, W, a):
    nc = build_program()
    in_maps = prepare_inputs(features, adj, W, a)
    res = run_bass_kernel_spmd(nc, in_maps, list(range(N_CORES)))
    out = np.concatenate(
        [res.results[c]["out"] for c in range(N_CORES)], axis=0)
    return out.astype(np.float32)


if __name__ == "__main__":
    rng = np.random.default_rng(0)
    features = rng.standard_normal((N, F_IN), dtype=np.float32)
    adj = (rng.integers(0, 2, size=(N, N))).astype(np.float32)
    W = (rng.standard_normal((K, F_IN, H), dtype=np.float32) * 0.118)
    a = (rng.standard_normal((K, 2 * H), dtype=np.float32) * 0.176)
    out = kernel(features=features, adj=adj, W=W, a=a)
    print("out", out.shape, out.dtype, np.abs(out).max())


# revision 20
# speedup vs baseline: 1.1011x; 1.1011x over previous
"""GAT layer (N=8192, F_IN=256, H=64 per head, K=8 heads) on 8 Trainium2 cores.

Strategy (row-sharding, fully data-parallel, no collectives):
  reference per head k:
    h   = features @ W[k]                      [N, H]
    wh1 = h @ a[k,:H]; wh2 = h @ a[k,H:]       [N]
    e   = leaky_relu(wh1[:,None] + wh2[None,:], 0.2)
    att = softmax(where(adj>0, e, -9e15), axis=1)
    out = elu(att @ h)

  Algebra: with s = wh1[i] + wh2[j], G1 = exp(0.8 wh1), G2 = exp(0.8 wh2),
  X2 = exp(wh2):
    exp(lrelu(s)) / exp(0.2 wh1) = max(G1_i, 1/G2_j) * X2_j
  The row factor exp(0.2 wh1) cancels in softmax.  X2_j is folded into the
  value matrix ON THE HOST: h2 = [h * X2 | X2] (the last column produces the
  softmax denominator).  adj in {0,1} multiplies exactly:
    u[j,i]   = max(G1_i, invG2_j) * adj[j,i]
    acc[c,i] = sum_j h2[j,c] * u[j,i]            (PE, PSUM accumulate over j)
    out[i]   = elu(acc[0:H,i] / acc[H,i])

  Per-device work per j-group g (128 j's) within a 4-head sweep:
    2x DVE tensor_scalar   z_k = max(g1b_k, invG2_k[p])          (~200 ns)
    2x ACT activation      z_k = relu(g1b_k - invG2_k[p])        (~1.1 us,
                           parallel engine; exact PE correction below)
    1x DVE fused mask      u[s] = z[s] * adj  over all 4 heads   (~2.2 us,
                           stride-0 broadcast adjacency, ONE instruction)
    8x PE matmul [65,512]  main accumulation, both i-blocks
    4x PE matmul [65,512]  ACT-head corrections  acc += (h2*invG2)^T @ adj
  Batching the mask into one instruction (and keeping per-instruction
  semaphore overhead low) is worth ~2x on hardware vs per-head ops.
  Two 4-head sweeps over the adjacency (PSUM holds 8 [65,512] f32 accs).
  h2/g1b/invG2 come from tiny host matmuls (O(N K F) host work).

Per-core layout ([j,i]-transposed tiles so contraction j sits on partitions):
  adjr [128, 64, 2, 512] bf16 : adjr[p,g,ib,i] = adj[r0+ib*512+i, g*128+p]
  g1b  [128, 8, 2, 512] bf16  : G1 rows broadcast across partitions
  ig2  [128, 8, 64] f32       : invG2[p,k,g] = exp(-0.8 wh2)[g*128+p, k]
  nig2 [128, 8, 64] f32       : -invG2 (ACT bias)
  h2   [128, 64, 8, 65] bf16  : value matrix, X2-folded + den column
  h2c  [128, 64, 4, 65] bf16  : h2 * invG2 for the ACT heads (1,3,5,7)
"""

import sys
import os

sys.path.insert(0, "/opt/trn_rl_repo")

import numpy as np
import ml_dtypes
from contextlib import ExitStack

import concourse.bass as bass
import concourse.tile as tile
from concourse import bacc, mybir
from concourse.bass_utils import run_bass_kernel_spmd

N = 8192
F_IN = 256
H = 64
K = 8
ALPHA = 0.2
N_CORES = 8
R = N // N_CORES          # 1024 rows per core
IB = 2                    # i-blocks per core (512 columns of out-rows each)
IW = R // IB              # 512, i-width per block
G = N // 128              # 64 j-groups of 128
G_SUB = 4                 # j-groups per adjacency DMA
HA = H + 1                # 65: head value columns + denominator column
HC = G // 4               # 16 j-groups per h2 DMA chunk

SWEEPS = ((0, 1, 2, 3), (4, 5, 6, 7))
ACT_SLOTS = (1, 3)        # per-sweep slots whose z runs on the ACT engine
ACT_HEADS = tuple(sw[s] for sw in SWEEPS for s in ACT_SLOTS)  # (1,3,5,7)

F32 = mybir.dt.float32
BF16 = mybir.dt.bfloat16
AX = mybir.AluOpType

_cached = {}


def build_program(loop_t=1, n_act=len(ACT_SLOTS), pool_mask=False, pool_z=False, skew=True, u_bufs=3, z_bufs=3, gsub=None):
    key = (loop_t, n_act, pool_mask, pool_z, skew, u_bufs, z_bufs, gsub)
    if key in _cached:
        return _cached[key]

    nc = bacc.Bacc("TRN2", target_bir_lowering=False, debug=False,
                   num_devices=N_CORES)

    adjr_d = nc.dram_tensor("adjr", [128, G, IB, IW], BF16, kind="ExternalInput").ap()
    g1b_d = nc.dram_tensor("g1b", [128, K, IB, IW], BF16, kind="ExternalInput").ap()
    ig2_d = nc.dram_tensor("ig2", [128, K, G], F32, kind="ExternalInput").ap()
    nig2_d = nc.dram_tensor("nig2", [128, K, G], F32, kind="ExternalInput").ap()
    h2_d = nc.dram_tensor("h2", [128, G, K, HA], BF16, kind="ExternalInput").ap()
    h2c_d = nc.dram_tensor("h2c", [128, G, 4, HA], BF16, kind="ExternalInput").ap()
    out_d = nc.dram_tensor("out", [R, K * H], F32, kind="ExternalOutput").ap()

    PAIR = IB * IW            # 1024
    act_slots = ACT_SLOTS[:n_act]
    gsub = gsub or G_SUB

    with tile.TileContext(nc) as tc:
        with ExitStack() as ctx:
            const = ctx.enter_context(tc.tile_pool(name="const", bufs=1))
            adj_pool = ctx.enter_context(tc.tile_pool(name="adj", bufs=2))
            z_pool = ctx.enter_context(tc.tile_pool(name="z", bufs=z_bufs))
            u_pool = ctx.enter_context(tc.tile_pool(name="u", bufs=u_bufs))
            stg_pool = ctx.enter_context(tc.tile_pool(name="stg", bufs=4))
            fin_pool = ctx.enter_context(tc.tile_pool(name="fin", bufs=2))
            acc_sb_pool = ctx.enter_context(tc.tile_pool(name="accsb", bufs=2))
            psum = ctx.enter_context(tc.tile_pool(name="psum", bufs=8, space="PSUM"))

            # ---- constants ----
            g1b_sb = const.tile([128, K, IB, IW], BF16)
            nc.sync.dma_start(g1b_sb[:], g1b_d[:])
            ig2_sb = const.tile([128, K, G], F32)
            nc.sync.dma_start(ig2_sb[:], ig2_d[:])
            nig2_sb = const.tile([128, K, G], F32)
            nc.sync.dma_start(nig2_sb[:], nig2_d[:])
            ident = const.tile([128, 128], F32)
            from concourse.masks import make_identity
            make_identity(nc, ident[:])

            h2_sb = []
            h2c_sb = []
            for c in range(4):
                t = const.tile([128, HC, K, HA], BF16, name=f"h2_{c}")
                nc.sync.dma_start(t[:], h2_d[:, c * HC:(c + 1) * HC, :, :])
                h2_sb.append(t)
                if act_slots:
                    t2 = const.tile([128, HC, 4, HA], BF16, name=f"h2c_{c}")
                    nc.sync.dma_start(t2[:], h2c_d[:, c * HC:(c + 1) * HC, :, :])
                    h2c_sb.append(t2)

            loop_cm = tc.For_i(0, loop_t, 1) if loop_t > 1 else None
            if loop_cm is not None:
                ctx.enter_context(loop_cm)

            for si, sweep_heads in enumerate(SWEEPS):
                accs = {}
                for k in sweep_heads:
                    for ib in range(IB):
                        accs[(k, ib)] = psum.tile(
                            [HA, IW], F32, tag="ps", name=f"acc{k}_{ib}")
                def g1b_of(k):
                    return g1b_sb[:, k, :, :].rearrange("p b i -> p (b i)")

                def emit_z(dst, s, k, g):
                    if s in act_slots:
                        nc.scalar.activation(
                            dst, g1b_of(k),
                            mybir.ActivationFunctionType.Relu,
                            bias=nig2_sb[:, k, g:g + 1])
                    else:
                        eng = nc.gpsimd if (pool_z and s == 0) else nc.vector
                        eng.tensor_scalar(
                            dst, g1b_of(k),
                            ig2_sb[:, k, g:g + 1], None, op0=AX.max)

                def emit_mask_mm(g, z_all, adj_t, gi):
                    adj_pair = adj_t[:, gi, :, :].rearrange("p b i -> p (b i)")
                    u_all = u_pool.tile([128, 4, PAIR], BF16, tag="u")
                    adj_bc = adj_pair.unsqueeze(1).broadcast_to((128, 4, PAIR))
                    nc.vector.tensor_tensor(u_all[:], z_all[:], adj_bc,
                                            op=AX.mult)
                    for s, k in enumerate(sweep_heads):
                        lhs = h2_sb[g // HC][:, g % HC, k, :]
                        last = (g == G - 1) and s not in act_slots
                        for ib in range(IB):
                            nc.tensor.matmul(
                                accs[(k, ib)][:],
                                lhs,
                                u_all[:, s, ib * IW:(ib + 1) * IW],
                                start=(g == 0),
                                stop=last,
                            )
                    # exact corrections for the ACT heads:
                    # acc += (h2*invG2)^T @ adj
                    for s in act_slots:
                        k = sweep_heads[s]
                        ci = si * len(ACT_SLOTS) + ACT_SLOTS.index(s)
                        lhsc = h2c_sb[g // HC][:, g % HC, ci, :]
                        for ib in range(IB):
                            nc.tensor.matmul(
                                accs[(k, ib)][:],
                                lhsc,
                                adj_t[:, gi, ib, :],
                                start=False,
                                stop=(g == G - 1),
                            )

                pending = None   # (g, z_all, adj_t, gi) awaiting mask+mm
                for gs in range(G // gsub):
                    adj_t = adj_pool.tile([128, gsub, IB, IW], BF16)
                    nc.sync.dma_start(
                        adj_t[:], adjr_d[:, gs * gsub:(gs + 1) * gsub, :, :]
                    )
                    for gi in range(gsub):
                        g = gs * gsub + gi
                        z_all = z_pool.tile([128, 4, PAIR], BF16, tag="z")
                        for s, k in enumerate(sweep_heads):
                            emit_z(z_all[:, s, :], s, k, g)
                        if skew:
                            if pending is not None:
                                emit_mask_mm(*pending)
                            pending = (g, z_all, adj_t, gi)
                        else:
                            emit_mask_mm(g, z_all, adj_t, gi)
                if pending is not None:
                    emit_mask_mm(*pending)
                    pending = None
                # ---- drain this sweep ----
                sw = list(sweep_heads)
                for ib in range(IB):
                    stgs = [stg_pool.tile([128, 4, HA], F32, tag="stg",
                                          name=f"stg{sw[0]}_{ib}_{c}")
                            for c in range(IW // 128)]
                    for s, k in enumerate(sw):
                        acc_sb = acc_sb_pool.tile([HA, IW], F32, tag="accsb")
                        nc.scalar.copy(acc_sb[:], accs[(k, ib)][:])
                        for c in range(IW // 128):
                            pst = psum.tile([128, HA], F32, tag="ps",
                                            name=f"pst{k}_{ib}_{c}")
                            nc.tensor.transpose(
                                pst[:], acc_sb[:, c * 128:(c + 1) * 128],
                                ident[0:HA, 0:HA],
                            )
                            nc.scalar.copy(stgs[c][:, s, :], pst[:])
                    for c in range(IW // 128):
                        stg = stgs[c]
                        recips = fin_pool.tile([128, 4], F32, tag="recip")
                        nc.vector.reciprocal(recips[:], stg[:, :, H])
                        fin = fin_pool.tile([128, 4, H], F32, tag="fin")
                        nc.vector.tensor_tensor(
                            fin[:], stg[:, :, 0:H],
                            recips[:].unsqueeze(2).broadcast_to((128, 4, H)),
                            op=AX.mult,
                        )
                        # elu(x) = exp(min(x,0)) + (max(x,0) - 1)
                        fin2 = fin_pool.tile([128, 4 * H], F32, tag="fin2")
                        finf = fin[:].rearrange("p k f -> p (k f)")
                        nc.vector.tensor_scalar(
                            fin2[:], finf, 0.0, None, op0=AX.min
                        )
                        ex = fin_pool.tile([128, 4 * H], F32, tag="ex")
                        nc.scalar.activation(
                            ex[:], fin2[:], mybir.ActivationFunctionType.Exp
                        )
                        rel = fin_pool.tile([128, 4 * H], F32, tag="rel")
                        nc.vector.tensor_scalar(
                            rel[:], finf, 0.0, -1.0, op0=AX.max, op1=AX.add
                        )
                        res = fin_pool.tile([128, 4 * H], F32, tag="res")
                        nc.vector.tensor_tensor(res[:], ex[:], rel[:], op=AX.add)
                        resv = res[:].rearrange("p (k f) -> p k f", k=4)
                        nc.sync.dma_start(
                            out_d[ib * IW + c * 128:
                                  ib * IW + (c + 1) * 128,
                                  sw[0] * H:(sw[-1] + 1) * H],
                            resv[:],
                        )

    nc.compile()
    _cached[key] = nc
    return nc


def prepare_inputs(features, adj, W, a):
    """Host-side prep: tiny projections + per-core sharded/transposed layouts."""
    features = np.asarray(features, dtype=np.float32)
    adj = np.asarray(adj, dtype=np.float32)
    W = np.asarray(W, dtype=np.float32)
    a = np.asarray(a, dtype=np.float32)

    # av[k] = W[k] @ a[k]  -> wh = features @ av.T   (tiny: K*F_IN*H flops)
    av1 = np.einsum("kfh,kh->kf", W, a[:, :H])          # [K, F_IN]
    av2 = np.einsum("kfh,kh->kf", W, a[:, H:])          # [K, F_IN]
    wh1 = features @ av1.T                               # [N, K]
    wh2 = features @ av2.T                               # [N, K]
    G1 = np.exp(0.8 * wh1).astype(np.float32)            # row factors
    IG2 = np.exp(-0.8 * wh2).astype(np.float32)          # 1/G2 col factors
    X2 = np.exp(wh2).astype(np.float32)                  # folded into values

    # h2[j, k, :] = [h_k[j] * X2[j,k] | X2[j,k]]
    h2full = np.empty((N, K, HA), dtype=np.float32)
    for k in range(K):
        hk = features @ W[k]                             # [N, H]
        h2full[:, k, 0:H] = hk * X2[:, k:k + 1]
        h2full[:, k, H] = X2[:, k]
    h2cfull = np.empty((N, 4, HA), dtype=np.float32)
    for ci, k in enumerate(ACT_HEADS):
        h2cfull[:, ci, :] = h2full[:, k, :] * IG2[:, k:k + 1]
    # [128, G, K, HA], partition p = j % 128 within group g = j // 128
    h2 = np.ascontiguousarray(
        h2full.reshape(G, 128, K, HA).transpose(1, 0, 2, 3)
    ).astype(ml_dtypes.bfloat16)
    h2c = np.ascontiguousarray(
        h2cfull.reshape(G, 128, 4, HA).transpose(1, 0, 2, 3)
    ).astype(ml_dtypes.bfloat16)

    # ig2[p,k,g] = IG2[g*128+p, k]
    ig2 = np.ascontiguousarray(
        IG2.reshape(G, 128, K).transpose(1, 2, 0))       # [128, K, G]
    nig2 = np.ascontiguousarray(-ig2)

    in_maps = []
    for c in range(N_CORES):
        r0 = c * R
        # adjr[p, g, ib, i] = adj[r0 + ib*IW + i, g*128 + p]
        blk = adj[r0:r0 + R, :]                          # [R, N]
        adj_r = np.ascontiguousarray(
            blk.reshape(IB, IW, G, 128).transpose(3, 2, 0, 1)
        ).astype(ml_dtypes.bfloat16)                     # [128, G, IB, IW]
        # g1b[p, k, ib, i] = G1[r0 + ib*IW + i, k]
        g1_blk = G1[r0:r0 + R, :].reshape(IB, IW, K).transpose(2, 0, 1)
        g1b = np.broadcast_to(
            g1_blk[None].astype(ml_dtypes.bfloat16), (128, K, IB, IW))
        g1b = np.ascontiguousarray(g1b)
        in_maps.append({
            "adjr": adj_r,
            "g1b": g1b,
            "ig2": ig2,
            "nig2": nig2,
            "h2": h2,
            "h2c": h2c,
        })
    return in_maps


def kernel(features, adj, W, a):
    nc = build_program()
    in_maps = prepare_inputs(features, adj, W, a)
    res = run_bass_kernel_spmd(nc, in_maps, list(range(N_CORES)))
    out = np.concatenate(
        [res.results[c]["out"] for c in range(N_CORES)], axis=0)
    return out.astype(np.float32)


if __name__ == "__main__":
    rng = np.random.default_rng(0)
    features = rng.standard_normal((N, F_IN), dtype=np.float32)
    adj = (rng.integers(0, 2, size=(N, N))).astype(np.float32)
    W = (rng.standard_normal((K, F_IN, H), dtype=np.float32) * 0.118)
    a = (rng.standard_normal((K, 2 * H), dtype=np.float32) * 0.176)
    out = kernel(features=features, adj=adj, W=W, a=a)
    print("out", out.shape, out.dtype, np.abs(out).max())
